# revision 2
# baseline (speedup 1.0000x reference)
"""Trainium2 Bass kernel for nn_ATTConv (per-node attention over 3 neighbor
aggregates + center, per-type Linear(2D->1) scorer, LeakyReLU, softmax,
weighted sum).

Sharding: data-parallel over the node axis B across 8 cores; per-type
attention weights replicated.

Layout: nodes on SBUF partitions (128 per subtile), D on the free axis.
  - score dots: DVE tensor_tensor mult + free-axis reduce
  - softmax: ScalarE Lrelu/Exp, DVE reduce + reciprocal
  - weighted aggregation: PE matmuls with diagonal lhsT = diag(attn_c),
    accumulated in PSUM (psum[n,d] += attn_c[n] * E_c[n,d])
  - PSUM -> SBUF via ScalarE copy, then DMA out
"""

import numpy as np

T = 3
B = 100000
D = 128
NCORES = 8
BS = B // NCORES  # 12500 rows per core per type
GROUP = 2048  # rows per processing group (16 subtiles of 128)

NEG_SLOPE = 0.01

_cache = {}


def _groups(total, group):
    """Split `total` rows into groups: full `group`-row groups, then a full
    128-subtile remainder group, then a final partial-partition group."""
    out = []
    r0 = 0
    while total - r0 >= group:
        out.append((r0, group))
        r0 += group
    rem = total - r0
    if rem >= 128:
        full = (rem // 128) * 128
        out.append((r0, full))
        r0 += full
        rem -= full
    if rem:
        out.append((r0, rem))
    return out


def build_nc(bs=BS, group=GROUP):
    import concourse.bacc as bacc
    import concourse.tile as tile
    from concourse import mybir
    import concourse.bass as bass
    from concourse.masks import make_identity

    f32 = mybir.dt.float32
    nc = bacc.Bacc("TRN2", target_bir_lowering=False, debug=False)

    hc = nc.dram_tensor("h_center", [T, bs, D], f32, kind="ExternalInput")
    hn = nc.dram_tensor("h_neigh", [T, T, bs, D], f32, kind="ExternalInput")
    aw = nc.dram_tensor("att_w", [T, 2 * D], f32, kind="ExternalInput")
    ab = nc.dram_tensor("att_b", [T], f32, kind="ExternalInput")
    out = nc.dram_tensor("out", [T, bs, D], f32, kind="ExternalOutput")

    NC = T + 1  # candidates: 3 neighbor types + center

    with tile.TileContext(nc) as tc:
        with (
            tc.tile_pool(name="const", bufs=1) as const,
            tc.tile_pool(name="ep", bufs=2) as ep,
            tc.tile_pool(name="tp", bufs=2) as tp,
            tc.tile_pool(name="sp", bufs=3) as sp,
            tc.tile_pool(name="dp", bufs=8) as dp,
            tc.tile_pool(name="op", bufs=2) as op,
            tc.tile_pool(name="pp", bufs=4, space="PSUM") as pp,
        ):
            # --- constants -------------------------------------------------
            ident = const.tile([128, 128], f32)
            make_identity(nc, ident[:, :])

            # W[p, t, c, d]: c in 0..3 -> w_e (second half of att_w row t),
            # c == 4 -> w_h (first half). Broadcast across partitions.
            W = const.tile([128, T, NC + 1, D], f32)
            aw_ap = aw.ap()
            for t in range(T):
                # w_e replicated into slots 0..3
                nc.gpsimd.dma_start(
                    out=W[:, t, 0:NC, :],
                    in_=bass.AP(
                        tensor=aw_ap.tensor,
                        offset=t * 2 * D + D,
                        ap=[[0, 128], [0, NC], [1, D]],
                    ),
                )
                # w_h in slot 4
                nc.gpsimd.dma_start(
                    out=W[:, t, NC, :],
                    in_=bass.AP(
                        tensor=aw_ap.tensor,
                        offset=t * 2 * D,
                        ap=[[0, 128], [1, D]],
                    ),
                )
            bias = const.tile([128, T], f32)
            nc.gpsimd.dma_start(
                out=bias[:, :],
                in_=bass.AP(tensor=ab.ap().tensor, offset=0, ap=[[0, 128], [1, T]]),
            )

            # --- main loop -------------------------------------------------
            for t in range(T):
                for r0, nr in _groups(bs, group):
                    S = (nr + 127) // 128  # subtiles in this group
                    pfull = nr % 128 == 0
                    plast = nr - (S - 1) * 128  # partitions in last subtile
                    assert pfull == (plast == 128)

                    # load candidate streams
                    etiles = []
                    for c in range(NC):
                        E = ep.tile([128, S, D], f32, tag=f"E{c}")
                        src = (
                            hn.ap()[t, c, r0 : r0 + nr, :]
                            if c < T
                            else hc.ap()[t, r0 : r0 + nr, :]
                        )
                        if S > 1 or pfull:
                            sf = S if pfull else S - 1
                            nc.sync.dma_start(
                                out=E[:, 0:sf, :],
                                in_=src[0 : sf * 128, :].rearrange(
                                    "(s p) d -> p s d", p=128
                                ),
                            )
                        if not pfull:
                            nc.sync.dma_start(
                                out=E[0:plast, S - 1, :],
                                in_=src[(S - 1) * 128 :, :].rearrange(
                                    "(s p) d -> p s d", p=plast
                                ),
                            )
                        etiles.append(E)

                    pmax = 128 if pfull else plast if S == 1 else 128
                    # NOTE: groups are constructed so that either all subtiles
                    # are full (pfull) or S == 1 (single partial subtile).
                    assert pfull or S == 1

                    # --- score dots ---------------------------------------
                    sc_e = sp.tile([128, S, NC], f32, tag="sc_e")
                    sc_h = sp.tile([128, S, 1], f32, tag="sc_h")
                    for c in range(NC):
                        tmp = tp.tile([128, S, D], f32, tag="tmp")
                        wb = W[0:pmax, t, c : c + 1, :].broadcast_to((pmax, S, D))
                        nc.vector.tensor_tensor(
                            out=tmp[0:pmax],
                            in0=etiles[c][0:pmax],
                            in1=wb,
                            op=mybir.AluOpType.mult,
                        )
                        nc.vector.tensor_reduce(
                            out=sc_e[0:pmax, :, c],
                            in_=tmp[0:pmax],
                            axis=mybir.AxisListType.X,
                            op=mybir.AluOpType.add,
                        )
                    # center dot with w_h
                    tmp = tp.tile([128, S, D], f32, tag="tmp")
                    wb = W[0:pmax, t, NC : NC + 1, :].broadcast_to((pmax, S, D))
                    nc.vector.tensor_tensor(
                        out=tmp[0:pmax],
                        in0=etiles[T][0:pmax],
                        in1=wb,
                        op=mybir.AluOpType.mult,
                    )
                    nc.vector.tensor_reduce(
                        out=sc_h[0:pmax, :, 0],
                        in_=tmp[0:pmax],
                        axis=mybir.AxisListType.X,
                        op=mybir.AluOpType.add,
                    )

                    # raw = sc_e + bias_t + sc_h (broadcast over candidates)
                    raw = sp.tile([128, S, NC], f32, tag="raw")
                    nc.vector.scalar_tensor_tensor(
                        out=raw[0:pmax],
                        in0=sc_e[0:pmax],
                        scalar=bias[0:pmax, t : t + 1],
                        in1=sc_h[0:pmax].broadcast_to((pmax, S, NC)),
                        op0=mybir.AluOpType.add,
                        op1=mybir.AluOpType.add,
                    )
                    # LeakyReLU(y) = (0.5+slope/2)*y + (0.5-slope/2)*|y|
                    ha = 0.5 + NEG_SLOPE / 2.0
                    hb = 0.5 - NEG_SLOPE / 2.0
                    absr = sp.tile([128, S, NC], f32, tag="absr")
                    nc.scalar.activation(
                        out=absr[0:pmax],
                        in_=raw[0:pmax],
                        func=mybir.ActivationFunctionType.Abs,
                        scale=hb,
                    )
                    leaky = sp.tile([128, S, NC], f32, tag="leaky")
                    nc.vector.scalar_tensor_tensor(
                        out=leaky[0:pmax],
                        in0=raw[0:pmax],
                        scalar=ha,
                        in1=absr[0:pmax],
                        op0=mybir.AluOpType.mult,
                        op1=mybir.AluOpType.add,
                    )
                    # ex = exp(leaky)  (no max-subtraction; scores are O(1))
                    ex = sp.tile([128, S, NC], f32, tag="ex")
                    nc.scalar.activation(
                        out=ex[0:pmax],
                        in_=leaky[0:pmax],
                        func=mybir.ActivationFunctionType.Exp,
                    )
                    ssum = sp.tile([128, S, 1], f32, tag="ssum")
                    nc.vector.tensor_reduce(
                        out=ssum[0:pmax, :, 0],
                        in_=ex[0:pmax],
                        axis=mybir.AxisListType.X,
                        op=mybir.AluOpType.add,
                    )
                    rcp = sp.tile([128, S, 1], f32, tag="rcp")
                    nc.vector.reciprocal(out=rcp[0:pmax], in_=ssum[0:pmax])

                    # --- aggregation via diagonal matmuls ------------------
                    out_sb = op.tile([128, S, D], f32, tag="out_sb")
                    for s4 in range(0, S, 4):
                        sn = min(4, S - s4)
                        ps = pp.tile([128, 4, D], f32, tag="ps")
                        for si in range(s4, s4 + sn):
                            for c in range(NC):
                                dg = dp.tile([128, 128], f32, tag="dg")
                                nc.vector.tensor_scalar(
                                    dg[0:pmax],
                                    ident[0:pmax],
                                    ex[0:pmax, si, c : c + 1],
                                    rcp[0:pmax, si, 0:1],
                                    mybir.AluOpType.mult,
                                    mybir.AluOpType.mult,
                                )
                                nc.tensor.matmul(
                                    ps[0:pmax, si - s4, :],
                                    dg[0:pmax, 0:pmax],
                                    etiles[c][0:pmax, si, :],
                                    start=(c == 0),
                                    stop=(c == NC - 1),
                                )
                        nc.scalar.activation(
                            out=out_sb[0:pmax, s4 : s4 + sn, :],
                            in_=ps[0:pmax, 0:sn, :],
                            func=mybir.ActivationFunctionType.Copy,
                        )
                    # store
                    if pfull:
                        nc.sync.dma_start(
                            out=out.ap()[t, r0 : r0 + nr, :].rearrange(
                                "(s p) d -> p s d", p=128
                            ),
                            in_=out_sb[:, 0:S, :],
                        )
                    else:
                        nc.sync.dma_start(
                            out=out.ap()[t, r0 : r0 + nr, :].rearrange(
                                "(s p) d -> p s d", p=plast
                            ),
                            in_=out_sb[0:plast, 0:S, :],
                        )

    nc.compile()
    return nc


def _get_nc():
    if "nc" not in _cache:
        _cache["nc"] = build_nc()
    return _cache["nc"]


def kernel(h_center, h_neigh, att_w, att_b):
    from concourse.bass_utils import run_bass_kernel_spmd

    nc = _get_nc()
    h_center = np.asarray(h_center, dtype=np.float32)
    h_neigh = np.asarray(h_neigh, dtype=np.float32)
    att_w = np.asarray(att_w, dtype=np.float32)
    att_b = np.asarray(att_b, dtype=np.float32)

    in_maps = []
    for c in range(NCORES):
        sl = slice(c * BS, (c + 1) * BS)
        in_maps.append(
            {
                "h_center": np.ascontiguousarray(h_center[:, sl, :]),
                "h_neigh": np.ascontiguousarray(h_neigh[:, :, sl, :]),
                "att_w": att_w,
                "att_b": att_b,
            }
        )
    res = run_bass_kernel_spmd(nc, in_maps, core_ids=list(range(NCORES)))
    return np.concatenate([r["out"] for r in res.results], axis=1)


# revision 28
# speedup vs baseline: 21.8231x; 21.8231x over previous
"""Trainium2 Bass kernel for nn_ATTConv (per-node attention over 3 neighbor
aggregates + center, per-type Linear(2D->1) scorer, LeakyReLU, softmax,
weighted sum).

Sharding: data-parallel over the node axis B across 8 cores; per-type
attention weights replicated. Inputs/outputs cross HBM as fp16 (host casts,
~8e-4 absmax-relative error); on-chip accumulation is fp32.

Layout: nodes on SBUF partitions (128 per subtile), D on the free axis.
Engine split per 512-node chunk:
  - all 5 score dots on PE: transpose the 4 candidate tiles -> PSUM,
    copy to SBUF fp16 (ScalarE + VectorE), 4 accumulating matmuls with
    selector-column weights (d on partitions) produce the [5, N] score
    rows in one PSUM tile, small transposes bring scores back node-major
  - softmax: ScalarE Abs/Exp (LeakyReLU = 0.505*y + 0.495*|y|), VectorE
    reduce + reciprocal
  - weighted aggregation: PE matmuls with diagonal lhsT = diag(attn_c)
    fp16 built on VectorE (tensor_scalar on an identity tile),
    accumulated in PSUM fp32 (psum[n,d] += attn_c[n] * E_c[n,d])
  - PSUM -> SBUF fp16 via VectorE copy, then DMA out
  - the <128-row tail group falls back to DVE mult+reduce dots
"""

import numpy as np

T = 3
B = 100000
D = 128
NCORES = 8
BS = B // NCORES  # 12500 rows per core per type
BSP = 12544  # padded to 98 full 128-row subtiles (pad rows are dropped)
GROUP = 1024  # rows per processing group (8 subtiles of 128)

NEG_SLOPE = 0.01

_cache = {}


def _groups(total, group):
    out = []
    r0 = 0
    while total - r0 >= group:
        out.append((r0, group))
        r0 += group
    rem = total - r0
    if rem >= 128:
        full = (rem // 128) * 128
        out.append((r0, full))
        r0 += full
        rem -= full
    if rem:
        out.append((r0, rem))
    return out


def build_nc(bs=BSP, group=GROUP, repeat=1, ep_bufs=10, sp_bufs=12, dp_bufs=16, pp_bufs=1, sct_bufs=1, et_bufs=4, pt_bufs=5, sc_bufs=1, op_bufs=4, ssp_bufs=4, mode="full"):
    import concourse.bacc as bacc
    import concourse.tile as tile
    from concourse import mybir
    import concourse.bass as bass
    from concourse.masks import make_identity

    f32 = mybir.dt.float32
    f16 = mybir.dt.float16
    nc = bacc.Bacc("TRN2", target_bir_lowering=False, debug=False)

    hc = nc.dram_tensor("h_center", [T, bs, D], f16, kind="ExternalInput")
    hn = nc.dram_tensor("h_neigh", [T, T, bs, D], f16, kind="ExternalInput")
    aw = nc.dram_tensor("att_w", [T, 2 * D], f32, kind="ExternalInput")
    ab = nc.dram_tensor("att_b", [T], f32, kind="ExternalInput")
    out = nc.dram_tensor("out", [T, bs, D], f16, kind="ExternalOutput")

    NC = T + 1  # candidates: 3 neighbor types + center

    with tile.TileContext(nc) as tc:
        with (
            tc.tile_pool(name="const", bufs=1) as const,
            tc.tile_pool(name="ep", bufs=ep_bufs) as ep,
            tc.tile_pool(name="tp", bufs=2) as tp,
            tc.tile_pool(name="sp", bufs=sp_bufs) as sp,
            tc.tile_pool(name="dp", bufs=dp_bufs) as dp,
            tc.tile_pool(name="op", bufs=op_bufs) as op,
            tc.tile_pool(name="etp", bufs=et_bufs) as etp,
            tc.tile_pool(name="ssp", bufs=ssp_bufs) as ssp,
            tc.tile_pool(name="ptps", bufs=pt_bufs, space="PSUM") as ptps,
            tc.tile_pool(name="scps", bufs=sc_bufs, space="PSUM") as scps,
            tc.tile_pool(name="sctps", bufs=sct_bufs, space="PSUM") as sctps,
            tc.tile_pool(name="pp", bufs=pp_bufs, space="PSUM") as pp,
        ):
            # --- constants -------------------------------------------------
            ident = const.tile([128, 128], f16)
            make_identity(nc, ident[:, :])
            identf = const.tile([8, 8], f32)
            make_identity(nc, identf[:, :])

            aw_ap = aw.ap()
            # wte_sel[d, t, k, m] fp16 selector weights (d on partitions):
            # k in 0..3: column m == k holds w_e(t) (candidate scores into
            # PSUM row k); k == 4: column 4 holds w_h(t) (center w_h score
            # into row 4). Other columns are zero so the 5 accumulating
            # matmuls write disjoint rows of one [5, N] PSUM tile (matmul
            # PSUM outputs must start at partition 0).
            NS = NC + 1  # 5 score rows
            # k in 0..2: column k holds w_e(t); k == 3 (the center matmul)
            # fills BOTH column 3 (w_e) and column 4 (w_h) so one pass over
            # the transposed center tile yields rows 3 and 4 together.
            wte_sel = const.tile([128, T, NC, NS], f16)
            nc.gpsimd.memset(wte_sel[:, :, :, :], 0.0)
            for t in range(T):
                for k in range(NC):
                    nc.gpsimd.dma_start(
                        out=wte_sel[:, t, k, k : k + 1],
                        in_=bass.AP(
                            tensor=aw_ap.tensor,
                            offset=t * 2 * D + D,
                            ap=[[1, 128], [1, 1]],
                        ),
                    )
                nc.gpsimd.dma_start(
                    out=wte_sel[:, t, NC - 1, NC : NC + 1],
                    in_=bass.AP(
                        tensor=aw_ap.tensor,
                        offset=t * 2 * D,
                        ap=[[1, 128], [1, 1]],
                    ),
                )
            bias = const.tile([128, T], f32)
            nc.gpsimd.dma_start(
                out=bias[:, :],
                in_=bass.AP(tensor=ab.ap().tensor, offset=0, ap=[[0, 128], [1, T]]),
            )

            # --- main loop -------------------------------------------------
            for _rep in range(repeat):
              for t in range(T):
                for r0, nr in _groups(bs, group):
                    S = (nr + 127) // 128
                    pfull = nr % 128 == 0
                    assert pfull, "bs must be a multiple of 128"
                    pmax = 128

                    # load candidate streams (fp16, S consecutive rows/part)
                    etiles = []
                    for c in range(NC):
                        E = ep.tile([128, S, D], f16, tag=f"E{c}")
                        src = (
                            hn.ap()[t, c, r0 : r0 + nr, :]
                            if c < T
                            else hc.ap()[t, r0 : r0 + nr, :]
                        )
                        nc.sync.dma_start(
                            out=E[:, :, :],
                            in_=src.rearrange("(p s) d -> p s d", p=128),
                        )
                        etiles.append(E)

                    # sc_e[:, :, 0:3]: neighbor scores (PE path)
                    # sc_e[:, :, 3]: center w_e score; sc_e[:, :, 4]: w_h
                    sc_e = sp.tile([128, S, NC + 1], f32, tag="sc_e")

                    # --- all 5 score dots on PE (per 512-node chunk) ------
                    if mode == "noscore":
                        nc.vector.memset(sc_e[0:pmax], 0.5)
                    elif mode == "dma":
                        pass
                    else:
                        for s4 in range(0, S, 4):
                            sn = min(4, S - s4)
                            sc = scps.tile([8, 4 * D], f32, tag="sc")
                            et = etp.tile([128, NC, 4, D], f16, tag="et")
                            for g in range(2):
                                pt = ptps.tile([128, 2, 4, D], f16, tag="pt")
                                for ci in range(2):
                                    c = 2 * g + ci
                                    for j in range(sn):
                                        nc.tensor.transpose(
                                            pt[:, ci, j, :],
                                            etiles[c][:, s4 + j, :],
                                            ident[:, :],
                                        )
                                if g == 0:
                                    nc.scalar.activation(
                                        out=et[:, 0:2, 0:sn, :],
                                        in_=pt[:, :, 0:sn, :],
                                        func=mybir.ActivationFunctionType.Copy,
                                    )
                                else:
                                    nc.scalar.activation(
                                        out=et[:, 2, 0:sn, :],
                                        in_=pt[:, 0, 0:sn, :],
                                        func=mybir.ActivationFunctionType.Copy,
                                    )
                                    nc.vector.tensor_copy(
                                        out=et[:, 3, 0:sn, :],
                                        in_=pt[:, 1, 0:sn, :],
                                    )
                            for k in range(NC):
                                nc.tensor.matmul(
                                    sc[0:NS, 0 : sn * D],
                                    wte_sel[:, t, k, :],
                                    et[:, k, 0:sn, :].rearrange(
                                        "p s d -> p (s d)"
                                    ),
                                    start=(k == 0),
                                    stop=(k == NC - 1),
                                )
                            ssb = ssp.tile([8, 4, D], f32, tag="ssb")
                            nc.scalar.activation(
                                out=ssb[0:NS, 0:sn, :],
                                in_=sc[0:NS, 0 : sn * D].rearrange(
                                    "c (s d) -> c s d", s=sn
                                ),
                                func=mybir.ActivationFunctionType.Copy,
                            )
                            sct = sctps.tile([128, 4, NS], f32, tag="sct")
                            for j in range(sn):
                                nc.tensor.transpose(
                                    sct[:, j, :],
                                    ssb[0:NS, j, :],
                                    identf[0:NS, 0:NS],
                                )
                            nc.scalar.activation(
                                out=sc_e[:, s4 : s4 + sn, :],
                                in_=sct[:, 0:sn, :],
                                func=mybir.ActivationFunctionType.Copy,
                            )
                    # raw = sc_e[:, :, 0:4] + bias_t + w_h-score (broadcast)
                    if mode == "dma":
                        out_sb = op.tile([128, S, D], f16, tag="out_sb")
                        nc.vector.memset(out_sb[0:pmax], 0.0)
                        nc.sync.dma_start(
                            out=out.ap()[t, r0 : r0 + nr, :].rearrange(
                                "(p s) d -> p s d", p=128
                            ),
                            in_=out_sb[:, 0:S, :],
                        )
                        continue
                    raw = sp.tile([128, S, NC], f32, tag="raw")
                    nc.vector.scalar_tensor_tensor(
                        out=raw[0:pmax],
                        in0=sc_e[0:pmax, :, 0:NC],
                        scalar=bias[0:pmax, t : t + 1],
                        in1=sc_e[0:pmax, :, NC : NC + 1].broadcast_to(
                            (pmax, S, NC)
                        ),
                        op0=mybir.AluOpType.add,
                        op1=mybir.AluOpType.add,
                    )
                    # LeakyReLU(y) = (0.5+slope/2)*y + (0.5-slope/2)*|y|
                    ha = 0.5 + NEG_SLOPE / 2.0
                    hb = 0.5 - NEG_SLOPE / 2.0
                    absr = sp.tile([128, S, NC], f32, tag="absr")
                    nc.scalar.activation(
                        out=absr[0:pmax],
                        in_=raw[0:pmax],
                        func=mybir.ActivationFunctionType.Abs,
                        scale=hb,
                    )
                    leaky = sp.tile([128, S, NC], f32, tag="leaky")
                    nc.vector.scalar_tensor_tensor(
                        out=leaky[0:pmax],
                        in0=raw[0:pmax],
                        scalar=ha,
                        in1=absr[0:pmax],
                        op0=mybir.AluOpType.mult,
                        op1=mybir.AluOpType.add,
                    )
                    ex = sp.tile([128, S, NC], f32, tag="ex")
                    nc.scalar.activation(
                        out=ex[0:pmax],
                        in_=leaky[0:pmax],
                        func=mybir.ActivationFunctionType.Exp,
                    )
                    ssum = sp.tile([128, S, 1], f32, tag="ssum")
                    nc.vector.tensor_reduce(
                        out=ssum[0:pmax, :, 0],
                        in_=ex[0:pmax],
                        axis=mybir.AxisListType.X,
                        op=mybir.AluOpType.add,
                    )
                    rcp = sp.tile([128, S, 1], f32, tag="rcp")
                    nc.vector.reciprocal(out=rcp[0:pmax], in_=ssum[0:pmax])

                    # --- aggregation via diagonal matmuls (fp16) -----------
                    out_sb = op.tile([128, S, D], f16, tag="out_sb")
                    if mode == "noagg":
                        nc.vector.tensor_scalar_mul(
                            out_sb[0:pmax], etiles[T][0:pmax], rcp[0:pmax, 0, 0:1]
                        )
                        nc.sync.dma_start(
                            out=out.ap()[t, r0 : r0 + nr, :].rearrange(
                                "(p s) d -> p s d", p=128
                            ),
                            in_=out_sb[:, 0:S, :],
                        )
                        continue
                    for s4 in range(0, S, 4):
                        sn = min(4, S - s4)
                        ps = pp.tile([128, 4, D], f32, tag="ps")
                        for si in range(s4, s4 + sn):
                            for c in range(NC):
                                dg = dp.tile([128, 128], f16, tag="dg")
                                nc.vector.tensor_scalar(
                                    dg[0:pmax],
                                    ident[0:pmax],
                                    ex[0:pmax, si, c : c + 1],
                                    rcp[0:pmax, si, 0:1],
                                    mybir.AluOpType.mult,
                                    mybir.AluOpType.mult,
                                )
                                nc.tensor.matmul(
                                    ps[0:pmax, si - s4, :],
                                    dg[0:pmax, 0:pmax],
                                    etiles[c][0:pmax, si, :],
                                    start=(c == 0),
                                    stop=(c == NC - 1),
                                )
                        nc.vector.tensor_copy(
                            out=out_sb[0:pmax, s4 : s4 + sn, :],
                            in_=ps[0:pmax, 0:sn, :],
                        )
                    # store
                    nc.sync.dma_start(
                        out=out.ap()[t, r0 : r0 + nr, :].rearrange(
                            "(p s) d -> p s d", p=128
                        ),
                        in_=out_sb[:, 0:S, :],
                    )

    nc.compile()
    return nc


def _get_nc():
    if "nc" not in _cache:
        _cache["nc"] = build_nc()
    return _cache["nc"]


def kernel(h_center, h_neigh, att_w, att_b):
    from concourse.bass_utils import run_bass_kernel_spmd

    nc = _get_nc()
    h_center = np.asarray(h_center, dtype=np.float32).astype(np.float16)
    h_neigh = np.asarray(h_neigh, dtype=np.float32).astype(np.float16)
    att_w = np.asarray(att_w, dtype=np.float32)
    att_b = np.asarray(att_b, dtype=np.float32)

    in_maps = []
    for c in range(NCORES):
        sl = slice(c * BS, (c + 1) * BS)
        hcp = np.zeros((T, BSP, D), np.float16)
        hcp[:, :BS] = h_center[:, sl, :]
        hnp = np.zeros((T, T, BSP, D), np.float16)
        hnp[:, :, :BS] = h_neigh[:, :, sl, :]
        in_maps.append(
            {"h_center": hcp, "h_neigh": hnp, "att_w": att_w, "att_b": att_b}
        )
    res = run_bass_kernel_spmd(nc, in_maps, core_ids=list(range(NCORES)))
    return np.concatenate(
        [r["out"][:, :BS].astype(np.float32) for r in res.results], axis=1
    )


# revision 29
# speedup vs baseline: 30.6910x; 1.4064x over previous
"""Trainium2 Bass kernel for nn_ATTConv (per-node attention over 3 neighbor
aggregates + center, per-type Linear(2D->1) scorer, LeakyReLU, softmax,
weighted sum).

Sharding: data-parallel over the node axis B across 8 cores; per-type
attention weights replicated. Inputs/outputs cross HBM as fp16 (host casts,
~8e-4 absmax-relative error); on-chip accumulation is fp32.

Layout: nodes on SBUF partitions (128 per subtile), D on the free axis.
Engine split per 512-node chunk:
  - all 5 score dots on PE: transpose the 4 candidate tiles -> PSUM,
    copy to SBUF fp16 (ScalarE + VectorE), 4 accumulating matmuls with
    selector-column weights (d on partitions) produce the [5, N] score
    rows in one PSUM tile, small transposes bring scores back node-major
  - softmax: ScalarE Abs/Exp (LeakyReLU = 0.505*y + 0.495*|y|), VectorE
    reduce + reciprocal
  - weighted aggregation: PE matmuls with diagonal lhsT = diag(attn_c)
    fp16 built on VectorE (tensor_scalar on an identity tile),
    accumulated in PSUM fp32 (psum[n,d] += attn_c[n] * E_c[n,d])
  - PSUM -> SBUF fp16 via VectorE copy, then DMA out
  - the <128-row tail group falls back to DVE mult+reduce dots
"""

import numpy as np

T = 3
B = 100000
D = 128
NCORES = 8
BS = B // NCORES  # 12500 rows per core per type
BSP = 12544  # padded to 98 full 128-row subtiles (pad rows are dropped)
GROUP = 1024  # rows per processing group (8 subtiles of 128)

NEG_SLOPE = 0.01

_cache = {}


def _groups(total, group):
    out = []
    r0 = 0
    while total - r0 >= group:
        out.append((r0, group))
        r0 += group
    rem = total - r0
    if rem >= 128:
        full = (rem // 128) * 128
        out.append((r0, full))
        r0 += full
        rem -= full
    if rem:
        out.append((r0, rem))
    return out


def build_nc(bs=BSP, group=GROUP, repeat=1, ep_bufs=10, sp_bufs=12, dp_bufs=16, pp_bufs=1, sct_bufs=1, et_bufs=4, pt_bufs=5, sc_bufs=1, op_bufs=4, ssp_bufs=4, mode="full"):
    import concourse.bacc as bacc
    import concourse.tile as tile
    from concourse import mybir
    import concourse.bass as bass
    from concourse.masks import make_identity

    f32 = mybir.dt.float32
    f16 = mybir.dt.float16
    nc = bacc.Bacc("TRN2", target_bir_lowering=False, debug=False)

    hc = nc.dram_tensor("h_center", [T, bs, D], f16, kind="ExternalInput")
    hn = nc.dram_tensor("h_neigh", [T, T, bs, D], f16, kind="ExternalInput")
    aw = nc.dram_tensor("att_w", [T, 2 * D], f32, kind="ExternalInput")
    ab = nc.dram_tensor("att_b", [T], f32, kind="ExternalInput")
    out = nc.dram_tensor("out", [T, bs, D], f16, kind="ExternalOutput")

    NC = T + 1  # candidates: 3 neighbor types + center

    with tile.TileContext(nc) as tc:
        with (
            tc.tile_pool(name="const", bufs=1) as const,
            tc.tile_pool(name="ep", bufs=ep_bufs) as ep,
            tc.tile_pool(name="tp", bufs=2) as tp,
            tc.tile_pool(name="sp", bufs=sp_bufs) as sp,
            tc.tile_pool(name="dp", bufs=dp_bufs) as dp,
            tc.tile_pool(name="op", bufs=op_bufs) as op,
            tc.tile_pool(name="etp", bufs=et_bufs) as etp,
            tc.tile_pool(name="ssp", bufs=ssp_bufs) as ssp,
            tc.tile_pool(name="ptps", bufs=pt_bufs, space="PSUM") as ptps,
            tc.tile_pool(name="scps", bufs=sc_bufs, space="PSUM") as scps,
            tc.tile_pool(name="sctps", bufs=sct_bufs, space="PSUM") as sctps,
            tc.tile_pool(name="pp", bufs=pp_bufs, space="PSUM") as pp,
        ):
            # --- constants -------------------------------------------------
            ident = const.tile([128, 128], f16)
            make_identity(nc, ident[:, :])
            identf = const.tile([8, 8], f32)
            make_identity(nc, identf[:, :])

            aw_ap = aw.ap()
            # wte_sel[d, t, k, m] fp16 selector weights (d on partitions):
            # k in 0..3: column m == k holds w_e(t) (candidate scores into
            # PSUM row k); k == 4: column 4 holds w_h(t) (center w_h score
            # into row 4). Other columns are zero so the 5 accumulating
            # matmuls write disjoint rows of one [5, N] PSUM tile (matmul
            # PSUM outputs must start at partition 0).
            NS = NC + 1  # 5 score rows
            # k in 0..2: column k holds w_e(t); k == 3 (the center matmul)
            # fills BOTH column 3 (w_e) and column 4 (w_h) so one pass over
            # the transposed center tile yields rows 3 and 4 together.
            wte_sel = const.tile([128, T, NC, NS], f16)
            nc.gpsimd.memset(wte_sel[:, :, :, :], 0.0)
            for t in range(T):
                for k in range(NC):
                    nc.gpsimd.dma_start(
                        out=wte_sel[:, t, k, k : k + 1],
                        in_=bass.AP(
                            tensor=aw_ap.tensor,
                            offset=t * 2 * D + D,
                            ap=[[1, 128], [1, 1]],
                        ),
                    )
                nc.gpsimd.dma_start(
                    out=wte_sel[:, t, NC - 1, NC : NC + 1],
                    in_=bass.AP(
                        tensor=aw_ap.tensor,
                        offset=t * 2 * D,
                        ap=[[1, 128], [1, 1]],
                    ),
                )
            bias = const.tile([128, T], f32)
            nc.gpsimd.dma_start(
                out=bias[:, :],
                in_=bass.AP(tensor=ab.ap().tensor, offset=0, ap=[[0, 128], [1, T]]),
            )

            # --- main loop -------------------------------------------------
            for _rep in range(repeat):
              for t in range(T):
                for r0, nr in _groups(bs, group):
                    S = (nr + 127) // 128
                    pfull = nr % 128 == 0
                    assert pfull, "bs must be a multiple of 128"
                    pmax = 128

                    # load candidate streams (fp16, S consecutive rows/part)
                    etiles = []
                    for c in range(NC):
                        E = ep.tile([128, S, D], f16, tag=f"E{c}")
                        src = (
                            hn.ap()[t, c, r0 : r0 + nr, :]
                            if c < T
                            else hc.ap()[t, r0 : r0 + nr, :]
                        )
                        nc.sync.dma_start(
                            out=E[:, :, :],
                            in_=src.rearrange("(p s) d -> p s d", p=128),
                        )
                        etiles.append(E)

                    # sc_e[:, :, 0:3]: neighbor scores (PE path)
                    # sc_e[:, :, 3]: center w_e score; sc_e[:, :, 4]: w_h
                    sc_e = sp.tile([128, S, NC + 1], f32, tag="sc_e")

                    # --- all 5 score dots on PE (per 512-node chunk) ------
                    if mode == "noscore":
                        nc.vector.memset(sc_e[0:pmax], 0.5)
                    elif mode == "dma":
                        pass
                    else:
                        for s4 in range(0, S, 4):
                            sn = min(4, S - s4)
                            sc = scps.tile([8, 4 * D], f32, tag="sc")
                            et = etp.tile([128, NC, 4, D], f16, tag="et")
                            for g in range(2):
                                pt = ptps.tile([128, 2, 4, D], f16, tag="pt")
                                for ci in range(2):
                                    c = 2 * g + ci
                                    for j in range(sn):
                                        nc.tensor.matmul(
                                            pt[:, ci, j, :],
                                            etiles[c][:, s4 + j, :],
                                            ident[:, :],
                                            is_transpose=True,
                                            start=True,
                                            stop=True,
                                        )
                                if g == 0:
                                    nc.scalar.activation(
                                        out=et[:, 0:2, 0:sn, :],
                                        in_=pt[:, :, 0:sn, :],
                                        func=mybir.ActivationFunctionType.Copy,
                                    )
                                else:
                                    nc.scalar.activation(
                                        out=et[:, 2, 0:sn, :],
                                        in_=pt[:, 0, 0:sn, :],
                                        func=mybir.ActivationFunctionType.Copy,
                                    )
                                    nc.vector.tensor_copy(
                                        out=et[:, 3, 0:sn, :],
                                        in_=pt[:, 1, 0:sn, :],
                                    )
                            for k in range(NC):
                                nc.tensor.matmul(
                                    sc[0:NS, 0 : sn * D],
                                    wte_sel[:, t, k, :],
                                    et[:, k, 0:sn, :].rearrange(
                                        "p s d -> p (s d)"
                                    ),
                                    start=(k == 0),
                                    stop=(k == NC - 1),
                                )
                            ssb = ssp.tile([8, 4, D], f32, tag="ssb")
                            nc.scalar.activation(
                                out=ssb[0:NS, 0:sn, :],
                                in_=sc[0:NS, 0 : sn * D].rearrange(
                                    "c (s d) -> c s d", s=sn
                                ),
                                func=mybir.ActivationFunctionType.Copy,
                            )
                            sct = sctps.tile([128, 4, NS], f32, tag="sct")
                            for j in range(sn):
                                nc.tensor.matmul(
                                    sct[:, j, :],
                                    ssb[0:NS, j, :],
                                    identf[0:NS, 0:NS],
                                    is_transpose=True,
                                    start=True,
                                    stop=True,
                                )
                            nc.scalar.activation(
                                out=sc_e[:, s4 : s4 + sn, :],
                                in_=sct[:, 0:sn, :],
                                func=mybir.ActivationFunctionType.Copy,
                            )
                    # raw = sc_e[:, :, 0:4] + bias_t + w_h-score (broadcast)
                    if mode == "dma":
                        out_sb = op.tile([128, S, D], f16, tag="out_sb")
                        nc.vector.memset(out_sb[0:pmax], 0.0)
                        nc.sync.dma_start(
                            out=out.ap()[t, r0 : r0 + nr, :].rearrange(
                                "(p s) d -> p s d", p=128
                            ),
                            in_=out_sb[:, 0:S, :],
                        )
                        continue
                    raw = sp.tile([128, S, NC], f32, tag="raw")
                    nc.vector.scalar_tensor_tensor(
                        out=raw[0:pmax],
                        in0=sc_e[0:pmax, :, 0:NC],
                        scalar=bias[0:pmax, t : t + 1],
                        in1=sc_e[0:pmax, :, NC : NC + 1].broadcast_to(
                            (pmax, S, NC)
                        ),
                        op0=mybir.AluOpType.add,
                        op1=mybir.AluOpType.add,
                    )
                    # LeakyReLU(y) = (0.5+slope/2)*y + (0.5-slope/2)*|y|
                    ha = 0.5 + NEG_SLOPE / 2.0
                    hb = 0.5 - NEG_SLOPE / 2.0
                    absr = sp.tile([128, S, NC], f32, tag="absr")
                    nc.scalar.activation(
                        out=absr[0:pmax],
                        in_=raw[0:pmax],
                        func=mybir.ActivationFunctionType.Abs,
                        scale=hb,
                    )
                    leaky = sp.tile([128, S, NC], f32, tag="leaky")
                    nc.vector.scalar_tensor_tensor(
                        out=leaky[0:pmax],
                        in0=raw[0:pmax],
                        scalar=ha,
                        in1=absr[0:pmax],
                        op0=mybir.AluOpType.mult,
                        op1=mybir.AluOpType.add,
                    )
                    ex = sp.tile([128, S, NC], f32, tag="ex")
                    nc.scalar.activation(
                        out=ex[0:pmax],
                        in_=leaky[0:pmax],
                        func=mybir.ActivationFunctionType.Exp,
                    )
                    ssum = sp.tile([128, S, 1], f32, tag="ssum")
                    nc.vector.tensor_reduce(
                        out=ssum[0:pmax, :, 0],
                        in_=ex[0:pmax],
                        axis=mybir.AxisListType.X,
                        op=mybir.AluOpType.add,
                    )
                    rcp = sp.tile([128, S, 1], f32, tag="rcp")
                    nc.vector.reciprocal(out=rcp[0:pmax], in_=ssum[0:pmax])

                    # --- aggregation via diagonal matmuls (fp16) -----------
                    out_sb = op.tile([128, S, D], f16, tag="out_sb")
                    if mode == "noagg":
                        nc.vector.tensor_scalar_mul(
                            out_sb[0:pmax], etiles[T][0:pmax], rcp[0:pmax, 0, 0:1]
                        )
                        nc.sync.dma_start(
                            out=out.ap()[t, r0 : r0 + nr, :].rearrange(
                                "(p s) d -> p s d", p=128
                            ),
                            in_=out_sb[:, 0:S, :],
                        )
                        continue
                    for s4 in range(0, S, 4):
                        sn = min(4, S - s4)
                        ps = pp.tile([128, 4, D], f32, tag="ps")
                        for si in range(s4, s4 + sn):
                            for c in range(NC):
                                dg = dp.tile([128, 128], f16, tag="dg")
                                nc.vector.tensor_scalar(
                                    dg[0:pmax],
                                    ident[0:pmax],
                                    ex[0:pmax, si, c : c + 1],
                                    rcp[0:pmax, si, 0:1],
                                    mybir.AluOpType.mult,
                                    mybir.AluOpType.mult,
                                )
                                nc.tensor.matmul(
                                    ps[0:pmax, si - s4, :],
                                    dg[0:pmax, 0:pmax],
                                    etiles[c][0:pmax, si, :],
                                    start=(c == 0),
                                    stop=(c == NC - 1),
                                )
                        nc.vector.tensor_copy(
                            out=out_sb[0:pmax, s4 : s4 + sn, :],
                            in_=ps[0:pmax, 0:sn, :],
                        )
                    # store
                    nc.sync.dma_start(
                        out=out.ap()[t, r0 : r0 + nr, :].rearrange(
                            "(p s) d -> p s d", p=128
                        ),
                        in_=out_sb[:, 0:S, :],
                    )

    nc.compile()
    return nc


def _get_nc():
    if "nc" not in _cache:
        _cache["nc"] = build_nc()
    return _cache["nc"]


def kernel(h_center, h_neigh, att_w, att_b):
    from concourse.bass_utils import run_bass_kernel_spmd

    nc = _get_nc()
    h_center = np.asarray(h_center, dtype=np.float32).astype(np.float16)
    h_neigh = np.asarray(h_neigh, dtype=np.float32).astype(np.float16)
    att_w = np.asarray(att_w, dtype=np.float32)
    att_b = np.asarray(att_b, dtype=np.float32)

    in_maps = []
    for c in range(NCORES):
        sl = slice(c * BS, (c + 1) * BS)
        hcp = np.zeros((T, BSP, D), np.float16)
        hcp[:, :BS] = h_center[:, sl, :]
        hnp = np.zeros((T, T, BSP, D), np.float16)
        hnp[:, :, :BS] = h_neigh[:, :, sl, :]
        in_maps.append(
            {"h_center": hcp, "h_neigh": hnp, "att_w": att_w, "att_b": att_b}
        )
    res = run_bass_kernel_spmd(nc, in_maps, core_ids=list(range(NCORES)))
    return np.concatenate(
        [r["out"][:, :BS].astype(np.float32) for r in res.results], axis=1
    )
